# revision 78
# baseline (speedup 1.0000x reference)
"""Trainium2 Bass kernel for nn_AttnBlock: GroupNorm -> single-head spatial
self-attention (QKV 1x1 convs, softmax over 1024 positions, AV) -> proj 1x1
conv -> residual.

Sharding: data-parallel over batch. B=16 -> 2 batches per NeuronCore x 8.

v2 design (vs the fp32r v1 baseline at ~150us single-pass):
  * QK fusion: biases q_b,k_b are zero, so scores = h^T (Wk^T Wq) h. The
    fused Wm = Wk^T Wq is computed host-side; the q- and k-projections
    collapse into one "t = Wm h" projection (saves 1 of 4 C x C GEMMs and
    all k-side evacuations). (Nonzero q_b folds into a per-key exp bias --
    query-only score terms are softmax-invariant and are dropped exactly.)
  * AV, softmax-denominator and proj matmuls run fp8 e4m3 with
    MatmulPerfMode.DoubleRow (2 fp8 weights per PE cell = 256-row virtual
    array, ~2x throughput). Operands live in pair-tiles [128, 2, free] so
    a [:, :, sl] slice is directly the DoubleRow [Ki, Ko=2, dim] AP.
  * The scores path (h, t = Wm h, QK^T) runs bf16: costs ~128 extra
    matmuls but doubles the margin vs the 2e-2 gate (9.4e-3 vs ~2e-2
    measured end-to-end). KERNEL_SCORES_DT=fp8 flips it to full-fp8.
  * Weights are pre-scaled by 16 on the host so their ~N(0, 1/512) entries
    sit in fp8's normal range; the inverse scaling folds for free into the
    exp activation (scores path) and the proj-evacuation multiplier.
  * Softmax needs no max-subtraction (logits ~N(0,1)); a fixed -2.0 bias
    inside the exp keeps e^s under fp8e4's 240 max. The denominator is an
    all-ones DoubleRow matmul over the quantized probs, so normalization is
    self-consistent with the quantization.
  * GN statistics run on a separate bf16 copy of x (half the lead-in DMA;
    h is quantized anyway); the fp32 x loads later, only for the residual.
    rstd uses a Newton-iterated fast-inverse-sqrt on DVE so ACT needs only
    Square/Identity/Exp -> a single activation-table load for the kernel.
    The per-subtile group-reduce + broadcast of the stats is one matmul
    against a block-diagonal averaging matrix.
  * Matmul accumulation chains for the two 512-wide chunks of an output
    tile are interleaved step-outer, so consecutive matmuls share their
    stationary operand (redundant LDWEIGHTS collapse; ~11% on HW).
  * Per-batch pipelining: batch b+1's x DMA + GN (DVE/ACT) hide under
    batch b's attention matmuls; outputs store per 512-chunk as soon as
    the residual add lands.
"""

import os
import sys

import numpy as np

for _p in ("/opt/trn_rl_repo", "/root/.axon_site/_ro/trn_rl_repo"):
    if os.path.isdir(_p) and _p not in sys.path:
        sys.path.insert(0, _p)

import concourse.bacc as bacc
import concourse.tile as tile
import concourse.mybir as mybir
from concourse.alu_op_type import AluOpType
from concourse.bass_utils import run_bass_kernel_spmd

B, C, H, W = 16, 512, 32, 32
N = H * W                  # 1024 spatial positions
GROUPS = 32
GS = C // GROUPS           # 16 channels per group
NCORES = 8
BPC = B // NCORES          # batches per core
CT = C // 128              # channel partition-subtiles (4)
NT = N // 128              # position partition-subtiles (8)
NCH = N // 512             # 512-wide free chunks (2)
EPS = 1e-5
ATTN_SCALE = float(C) ** -0.5
WSCALE = 16.0              # host-side weight pre-scale into fp8 normal range
EXP_BIAS = -2.0            # keeps exp(logit) <= e^3.4 ~ 30 < 240 (fp8e4 max)

F32 = mybir.dt.float32
F8 = mybir.dt.float8e4
BF16 = mybir.dt.bfloat16
DR = mybir.MatmulPerfMode.DoubleRow
Act = mybir.ActivationFunctionType
# scores-path dtype: the GN output h, the fused t = (Wk^T Wq) h projection,
# and the QK^T matmul. bf16 costs ~64 extra matmuls but doubles the accuracy
# margin vs the 2e-2 gate (9.7e-3 vs 1.58e-2 measured end-to-end); the
# AV/proj side stays fp8 DoubleRow either way.
SC_DT = {"bf16": BF16, "fp8": F8}[os.environ.get("KERNEL_SCORES_DT", "bf16")]
SC_DR = SC_DT == F8

LAST_RESULTS = None        # BassKernelResults of the most recent run (for test.py)

_PROGRAM_CACHE = {}


def _build_program(flags, loop_reps=None):
    """Build the per-core Bass program. flags = (qb_nz, vb_nz, pb_nz).

    loop_reps: if set, wrap the whole per-core body in a hardware For_i loop
    (benchmarking only -- output identical each rep since xs is re-read)."""
    qb_nz, vb_nz, pb_nz = flags
    nc = bacc.Bacc(
        "TRN2",
        target_bir_lowering=False,
        debug=False,
        enable_asserts=False,
        num_devices=NCORES,
    )

    def din(name, shape, dt=F32):
        return nc.dram_tensor(name, shape, dt, kind="ExternalInput").ap()

    xs16 = din("xs16", [BPC, CT, 128, N], BF16)
    gmat_d = din("gmat", [128, 128])
    gnwb_d = din("gnwb", [128, 2 * CT])   # gn_w cols [0:CT], gn_b cols [CT:2CT]
    wm_d = din("wm", [128, CT, C], SC_DT)  # fused (Wk^T Wq)^T, stacked subtiles
    wv_d = din("wv", [128, CT, C], SC_DT)
    wp_d = din("wp", [128, CT, C], F8)
    ones_d = din("ones", [128, 2, 128], F8)
    uq_d = din("uq", [128, CT, 1], SC_DT) if qb_nz else None
    vb_d = din("vb", [128, C]) if vb_nz else None
    pb_d = din("pb", [CT, 128, 1]) if pb_nz else None

    # bf16 output (host upcasts): halves the store bandwidth in the tail
    out_d = nc.dram_tensor("out", [BPC, CT, 128, N], BF16, kind="ExternalOutput").ap()

    with tile.TileContext(nc) as tc:
        _emit(tc, xs16, gmat_d, gnwb_d, wm_d, wv_d, wp_d, ones_d,
              uq_d, vb_d, pb_d, out_d, loop_reps=loop_reps)
    nc.compile()
    return nc


def _emit(tc, xs16, gmat_d, gnwb_d, wm_d, wv_d, wp_d, ones_d,
          uq_d, vb_d, pb_d, out_d, loop_reps=None):
    nc = tc.nc
    from contextlib import ExitStack
    ctx = ExitStack()
    with ctx:
        consts = ctx.enter_context(tc.tile_pool(name="consts", bufs=1))
        # operand tiles are allocated per DoubleRow PAIR ([128, 2, X]) rather
        # than one stacked [128, CT, X] tile: the Tile framework's dependency
        # tracking is tile-granular, so pair-tiles let a consumer's first
        # accumulation step start as soon as its own pair is written instead
        # of waiting for the whole stack
        xin = ctx.enter_context(tc.tile_pool(name="xin", bufs=2 * CT))
        xin16 = ctx.enter_context(tc.tile_pool(name="xin16", bufs=4 * CT))
        scr = ctx.enter_context(tc.tile_pool(name="scr", bufs=4))
        small = ctx.enter_context(tc.tile_pool(name="small", bufs=24))
        h8p = ctx.enter_context(tc.tile_pool(name="h8p", bufs=4))
        t8p = ctx.enter_context(tc.tile_pool(name="t8p", bufs=4))
        v8p = ctx.enter_context(tc.tile_pool(name="v8p", bufs=8))
        a8p = ctx.enter_context(tc.tile_pool(name="a8p", bufs=8))
        h28p = ctx.enter_context(tc.tile_pool(name="h28p", bufs=4))
        rpool = ctx.enter_context(tc.tile_pool(name="rpool", bufs=4))
        psmain = ctx.enter_context(tc.tile_pool(name="psmain", bufs=7, space="PSUM"))
        psgn = ctx.enter_context(tc.tile_pool(name="psgn", bufs=1, space="PSUM"))

        # ---- constants: GN-related + batch-0 x go first in the DMA queue ----
        def load_const(tag, src, shape, dt=F32):
            t = consts.tile(shape, dt, tag=tag)
            nc.sync.dma_start(out=t, in_=src)
            return t

        def load_x16(b):
            """bf16 x: GN input AND residual source. x is never needed in
            fp32 -- h is quantized to SC_DT anyway and the bf16 residual
            rounding (~1e-3 of output scale) is well inside the error
            budget. Halves the input DMA and lead-in."""
            xt = []
            for t in range(CT):
                a = xin16.tile([128, N], BF16, tag="x16")
                nc.sync.dma_start(out=a, in_=xs16[b, t])
                xt.append(a)
            return xt

        x16s = {}
        if loop_reps is None:
            x16s[0] = load_x16(0)

        gnwb_sb = load_const("gnwb", gnwb_d, [128, 2 * CT])
        gmat_sb = load_const("gmat", gmat_d, [128, 128])
        pb_sb = [load_const(f"pb{ci}", pb_d[ci], [128, 1]) for ci in range(CT)] if pb_d is not None else None

        wm_sb = load_const("wm", wm_d, [128, CT, C], SC_DT)
        wv_sb = load_const("wv", wv_d, [128, CT, C], SC_DT)
        ones_sb = load_const("ones", ones_d, [128, 2, 128], F8)
        wp_sb = load_const("wp", wp_d, [128, CT, C], F8)
        uq_sb = load_const("uq", uq_d, [128, CT, 1], SC_DT) if uq_d is not None else None
        vb_sb = load_const("vb", vb_d, [128, C]) if vb_d is not None else None

        ebias_sb = consts.tile([128, 1], F32, tag="ebias")
        nc.vector.memset(ebias_sb, EXP_BIAS)
        magic_sb = consts.tile([128, CT], mybir.dt.uint32, tag="magic")
        nc.vector.memset(magic_sb, 0x5F3759DF)

        # PE warmup: the first real matmul lands ~8us in (after x DMA + GN),
        # by which point the HAM clock gate has re-throttled the array to
        # 1.2 GHz. Dummy matmuls on a memset tile during the otherwise-idle
        # lead-in keep the PE busy so real work starts at full clock.
        warm_sb = consts.tile([128, 2, 512], F8, tag="warm")
        nc.vector.memset(warm_sb, 1.0)
        # trigger the one activation-table load (~1.3us) before x arrives,
        # instead of on the first GN Square inside the critical path
        act_warm = consts.tile([128, 1], F32, tag="actwarm")
        nc.scalar.activation(act_warm, ebias_sb, Act.Exp)
        def warmup(n_mm):
            wps = psmain.tile([128, 512], F32, tag="ps", name="wps")
            for i in range(n_mm):
                nc.tensor.matmul(wps, lhsT=warm_sb[:, :, :128], rhs=warm_sb,
                                 start=(i == 0), stop=(i == n_mm - 1),
                                 perf_mode=DR)

        def group_norm(xt):
            """GN over one batch -> h as CT//2 pair-tiles [128, 2, N].

            Half-tile sums feed one gmat matmul (group-reduce + broadcast for
            all subtiles at once); the scalar tail -- including rstd via
            Newton-iterated fast-inverse-sqrt on DVE -- runs vectorized on
            [128, CT] so ACT only ever needs Square/Copy/Exp (one table set).
            """
            hp = [h8p.tile([128, 2, N], SC_DT, tag="h8", name=f"h8_{j}")
                  for j in range(CT // 2)]
            pst = small.tile([128, 2 * CT], F32, tag="pst")
            for t in range(CT):
                nc.vector.reduce_sum(pst[:, 2*t:2*t+1], xt[t],
                                     mybir.AxisListType.X)
                sq = scr.tile([128, N], F32, tag="scr")
                nc.scalar.activation(sq, xt[t], Act.Square,
                                     accum_out=pst[:, 2*t+1:2*t+2])
            gps = psgn.tile([128, 2 * CT], F32, tag="gn")
            nc.tensor.matmul(gps, lhsT=gmat_sb, rhs=pst, start=True, stop=True)
            # stage stats into SBUF (DVE may read at most one PSUM operand)
            st = small.tile([128, 2 * CT], F32, tag="gst")
            nc.vector.tensor_copy(out=st, in_=gps)
            mean, ex2 = st[:, 0::2], st[:, 1::2]
            va = small.tile([128, CT], F32, tag="va")
            nc.vector.tensor_tensor(va, mean, mean, AluOpType.mult)
            nc.vector.tensor_tensor(va, ex2, va, AluOpType.subtract)
            nc.vector.tensor_scalar_add(va, va, EPS)
            y = small.tile([128, CT], F32, tag="y")
            yu = y.bitcast(mybir.dt.uint32)
            nc.vector.tensor_scalar(yu, va.bitcast(mybir.dt.uint32), 1, None,
                                    op0=AluOpType.logical_shift_right)
            nc.vector.tensor_tensor(yu, magic_sb, yu, AluOpType.subtract)
            w = small.tile([128, CT], F32, tag="w")
            # one Newton step: rstd rel err ~1.7e-3, negligible vs fp8 noise
            for _ in range(1):
                nc.vector.tensor_tensor(w, y, y, AluOpType.mult)
                nc.vector.tensor_tensor(w, w, va, AluOpType.mult)
                nc.vector.tensor_scalar(w, w, -0.5, 1.5,
                                        op0=AluOpType.mult, op1=AluOpType.add)
                nc.vector.tensor_tensor(y, y, w, AluOpType.mult)
            s = small.tile([128, CT], F32, tag="s")
            nc.vector.tensor_tensor(s, y, gnwb_sb[:, :CT], AluOpType.mult)
            bp = small.tile([128, CT], F32, tag="bp")
            nc.vector.tensor_tensor(bp, mean, s, AluOpType.mult)
            nc.vector.tensor_tensor(bp, gnwb_sb[:, CT:], bp, AluOpType.subtract)
            for t in range(CT):
                # normalize split across DVE and ACT (activation takes
                # per-partition scale/bias APs) to shorten the serial chain
                if t % 2 == 0:
                    nc.vector.tensor_scalar(hp[t // 2][:, t % 2, :], xt[t],
                                            s[:, t:t+1], bp[:, t:t+1],
                                            op0=AluOpType.mult, op1=AluOpType.add)
                else:
                    nc.scalar.activation(hp[t // 2][:, t % 2, :], xt[t], Act.Identity,
                                         bias=bp[:, t:t+1], scale=s[:, t:t+1])
            return hp

        def pair(op, i, sl):
            """[128, 2, sl] DoubleRow pair view of operand pair-list/const."""
            return op[i][:, :, sl] if isinstance(op, list) else op[:, 2*i:2*i+2, sl]

        def sub(op, i, sl):
            """[128, sl] single-subtile view of operand pair-list/const."""
            return (op[i // 2][:, i % 2, sl] if isinstance(op, list)
                    else op[:, i, sl])

        def contract(ps, lt, rt, lsl, rsl, nsub, dr):
            """Accumulate ps += sum_i lt_i[lsl].T @ rt_i[rsl] over subtiles,
            as DoubleRow pairs when dr (fp8) else one 128-row step each."""
            if dr:
                for i in range(nsub // 2):
                    nc.tensor.matmul(ps, lhsT=pair(lt, i, lsl),
                                     rhs=pair(rt, i, rsl),
                                     start=(i == 0), stop=(i == nsub // 2 - 1),
                                     perf_mode=DR)
            else:
                for i in range(nsub):
                    nc.tensor.matmul(ps, lhsT=sub(lt, i, lsl), rhs=sub(rt, i, rsl),
                                     start=(i == 0), stop=(i == nsub - 1))

        def contract_nch(psl_, lt, rt, lsl, rsls, nsub, dr):
            """Two interleaved accumulation chains (one per 512-chunk) with
            the step loop OUTER: consecutive matmuls share the same
            stationary operand, so redundant LDWEIGHTS can collapse."""
            if dr:
                for i in range(nsub // 2):
                    for ps, rsl in zip(psl_, rsls):
                        nc.tensor.matmul(ps, lhsT=pair(lt, i, lsl),
                                         rhs=pair(rt, i, rsl),
                                         start=(i == 0),
                                         stop=(i == nsub // 2 - 1),
                                         perf_mode=DR)
            else:
                for i in range(nsub):
                    for ps, rsl in zip(psl_, rsls):
                        nc.tensor.matmul(ps, lhsT=sub(lt, i, lsl),
                                         rhs=sub(rt, i, rsl),
                                         start=(i == 0), stop=(i == nsub - 1))

        def emit_tv(b, xt, h8):
            """t = (Wk^T Wq) h and vT projections for batch b."""
            t8 = [t8p.tile([128, 2, N], SC_DT, tag="t8", name=f"t8_{j}")
                  for j in range(CT // 2)]
            chks = [slice(512 * n, 512 * (n + 1)) for n in range(NCH)]
            for dt_ in range(CT):
                dsl = slice(128 * dt_, 128 * (dt_ + 1))
                pss = [psmain.tile([128, 512], F32, tag="ps", name=f"ps{_n}") for _n in range(NCH)]
                contract_nch(pss, wm_sb, h8, dsl, chks, CT, SC_DR)
                for nch in range(NCH):
                    nc.scalar.copy(out=t8[dt_ // 2][:, dt_ % 2, chks[nch]],
                                   in_=pss[nch])

            # ---- vT[key_sub, c] via h as stationary ----
            v8 = [v8p.tile([128, 2, C], F8, tag="v8", name=f"v8_{j}")
                  for j in range(NT // 2)]
            for nt_ in range(NT):
                psl = slice(128 * nt_, 128 * (nt_ + 1))
                ps = psmain.tile([128, 512], F32, tag="ps")
                contract(ps, h8, wv_sb, psl, slice(None), CT, SC_DR)
                vdst = v8[nt_ // 2][:, nt_ % 2, :]
                if vb_sb is not None:
                    nc.vector.tensor_tensor(vdst, ps, vb_sb, AluOpType.add)
                else:
                    nc.vector.tensor_copy(out=vdst, in_=ps)
            return t8, v8

        def emit_attn(b, xt, h8, t8, v8):
            # ---- scoresT + exp: a8[key_sub, q] fp8 probs ----
            # psum = 16*s_raw; logits = s_raw*C^-0.5; exp(logits - 2) via
            # activation scale+bias, output quantized to fp8
            a8 = [a8p.tile([128, 2, N], F8, tag="a8", name=f"a8_{j}")
                  for j in range(NT // 2)]
            if uq_sb is not None:
                sbias = {}
                for kt in range(NT):
                    ksl = slice(128 * kt, 128 * (kt + 1))
                    psb = psgn.tile([128, 1], F32, tag="gn")
                    for i in range(CT):
                        nc.tensor.matmul(psb, lhsT=sub(h8, i, ksl),
                                         rhs=uq_sb[:, i, :],
                                         start=(i == 0), stop=(i == CT - 1))
                    bt = small.tile([128, 1], F32, tag="bt")
                    nc.vector.tensor_scalar(bt, psb, ATTN_SCALE / WSCALE, EXP_BIAS,
                                            op0=AluOpType.mult, op1=AluOpType.add)
                    sbias[kt] = bt
            chks = [slice(512 * n, 512 * (n + 1)) for n in range(NCH)]
            for kt in range(NT):
                ksl = slice(128 * kt, 128 * (kt + 1))
                pss = [psmain.tile([128, 512], F32, tag="ps", name=f"ps{_n}") for _n in range(NCH)]
                contract_nch(pss, h8, t8, ksl, chks, CT, SC_DR)
                bias_arg = sbias[kt] if uq_sb is not None else ebias_sb
                for nch in range(NCH):
                    nc.scalar.activation(a8[kt // 2][:, kt % 2, chks[nch]],
                                         pss[nch], Act.Exp,
                                         bias=bias_arg,
                                         scale=ATTN_SCALE / WSCALE)

            # ---- softmax denominator from the quantized probs ----
            recips = []
            for nch in range(NCH):
                qsl = slice(512 * nch, 512 * (nch + 1))
                ps = psmain.tile([128, 512], F32, tag="ps")
                for i in range(NT // 2):
                    nc.tensor.matmul(ps, lhsT=ones_sb,
                                     rhs=a8[i][:, :, qsl],
                                     start=(i == 0), stop=(i == NT // 2 - 1),
                                     perf_mode=DR)
                rc = rpool.tile([128, 512], F32, tag="rc")
                nc.vector.reciprocal(out=rc, in_=ps)
                recips.append(rc)

            # ---- AV -> h2 (normalized, fp8, carries the 16x of v) ----
            h28 = [h28p.tile([128, 2, N], F8, tag="h28", name=f"h28_{j}")
                   for j in range(CT // 2)]
            for ct_ in range(CT):
                csl = slice(128 * ct_, 128 * (ct_ + 1))
                pss = [psmain.tile([128, 512], F32, tag="ps", name=f"ps{_n}") for _n in range(NCH)]
                contract_nch(pss, v8, a8, csl, chks, NT, True)
                for nch in range(NCH):
                    nc.vector.tensor_tensor(h28[ct_ // 2][:, ct_ % 2, chks[nch]],
                                            pss[nch], recips[nch], AluOpType.mult)

            # ---- proj + residual + chunked store (1/256 undoes 16x*16x) ----
            for dt_ in range(CT):
                dsl = slice(128 * dt_, 128 * (dt_ + 1))
                pss = [psmain.tile([128, 512], F32, tag="ps", name=f"ps{_n}") for _n in range(NCH)]
                contract_nch(pss, wp_sb, h28, dsl, chks, CT, True)
                for nch in range(NCH):
                    qsl = chks[nch]
                    ps = pss[nch]
                    xdst = xt[dt_][:, qsl]
                    if pb_sb is not None:
                        tproj = scr.tile([128, 512], F32, tag="scr")
                        nc.vector.tensor_scalar(tproj, ps,
                                                1.0 / (WSCALE * WSCALE), pb_sb[dt_],
                                                op0=AluOpType.mult, op1=AluOpType.add)
                        nc.vector.tensor_tensor(xdst, tproj, xdst, AluOpType.add)
                    elif nch == 1 and dt_ < 2:
                        # split the evac: ACT rescales out of PSUM, the idle
                        # GpSimd does the SBUF-side residual add. Only for
                        # EARLY chunks -- GpSimd's slow adds hide under the
                        # remaining proj matmuls, while the tail-critical
                        # last chunks stay on the faster DVE
                        tproj = scr.tile([128, 512], F32, tag="scr")
                        nc.scalar.activation(tproj, ps, Act.Identity,
                                             scale=1.0 / (WSCALE * WSCALE))
                        nc.gpsimd.tensor_tensor(xdst, tproj, xdst, AluOpType.add)
                    else:
                        nc.vector.scalar_tensor_tensor(
                            out=xdst, in0=ps,
                            scalar=1.0 / (WSCALE * WSCALE),
                            in1=xdst,
                            op0=AluOpType.mult, op1=AluOpType.add)
                # one whole-tile store per dt_: HWDGE descriptor issue
                # (~700ns each) dominates the bf16 store stream, so fewer
                # bigger stores shorten the tail
                nc.sync.dma_start(out=out_d[b, dt_], in_=xt[dt_])

        def body():
            # pipelined emission: batch b+1's x-loads + GN are queued between
            # batch b's t/v projections and its attention, so GN(b+1)'s
            # DVE/ACT work hides under batch b's long PE stretch
            if 0 not in x16s:
                # loop mode: loads must be emitted inside the For_i body so
                # every iteration re-reads x into the tiles this body consumes
                x16s[0] = load_x16(0)
            warmup(20)
            x160 = x16s.pop(0)
            state = [(x160, group_norm(x160))]
            for b in range(1, BPC):
                t8, v8 = emit_tv(b - 1, *state[b - 1])
                x16b = load_x16(b)
                state.append((x16b, group_norm(x16b)))
                emit_attn(b - 1, *state[b - 1], t8, v8)
            t8, v8 = emit_tv(BPC - 1, *state[BPC - 1])
            emit_attn(BPC - 1, *state[BPC - 1], t8, v8)

        if loop_reps is None:
            body()
        else:
            with tc.For_i(0, loop_reps, 1):
                body()


def _prep_inputs(x, gn_w, gn_b, q_w, q_b, k_w, k_b, v_w, v_b, p_w, p_b):
    f = np.float32
    f8 = mybir.dt.np(F8)
    scnp = mybir.dt.np(SC_DT)

    def pack_w(w64, npdt=f8):
        # w64: [D, C] output-major weight; stationary layout [128, CT, D]
        # stacked so [:, 2i:2i+2, :] is a DoubleRow [Ki, Ko=2, dim] pair
        wT = np.asarray(w64).T * WSCALE                       # [C, D]
        arr = wT.reshape(CT, 128, wT.shape[1]).transpose(1, 0, 2)
        return np.ascontiguousarray(np.clip(arr, -240.0, 240.0)).astype(npdt)

    x16 = np.ascontiguousarray(
        np.asarray(x, f).reshape(B, CT, 128, N).astype(mybir.dt.np(BF16)))
    q64 = np.asarray(q_w, np.float64)
    k64 = np.asarray(k_w, np.float64)
    wm = k64.T @ q64                                          # scores = h^T wm h
    base = {
        "gmat": np.ascontiguousarray(
            np.kron(np.eye(128 // GS, dtype=f), np.ones((GS, GS), f)) / (GS * N)),
        "gnwb": np.ascontiguousarray(np.concatenate(
            [np.asarray(gn_w, f).reshape(CT, 128).T,
             np.asarray(gn_b, f).reshape(CT, 128).T], axis=1)),
        "wm": pack_w(wm, scnp),
        "wv": pack_w(np.asarray(v_w, np.float64), scnp),
        "wp": pack_w(np.asarray(p_w, np.float64)),
        "ones": np.ones((128, 2, 128), f).astype(f8),
    }
    qb_nz = bool(np.any(np.asarray(q_b)))
    vb_nz = bool(np.any(np.asarray(v_b)))
    pb_nz = bool(np.any(np.asarray(p_b)))
    if qb_nz:
        # after softmax, only the key-dependent score term matters:
        # s[key,q] += (q_b^T Wk) h[:,key];  uq = Wk^T q_b, scaled like weights
        uq = (k64.T @ np.asarray(q_b, np.float64)) * WSCALE
        base["uq"] = np.ascontiguousarray(
            np.clip(uq.reshape(CT, 128, 1).transpose(1, 0, 2), -240.0, 240.0)
        ).astype(scnp)
    if vb_nz:
        base["vb"] = np.ascontiguousarray(
            np.broadcast_to(np.asarray(v_b, f)[None, :] * WSCALE, (128, C)).copy())
    if pb_nz:
        base["pb"] = np.ascontiguousarray(np.asarray(p_b, f)).reshape(CT, 128, 1)
    return x16, base, (qb_nz, vb_nz, pb_nz)


def kernel(x, temb, gn_w, gn_b, q_w, q_b, k_w, k_b, v_w, v_b, p_w, p_b):
    global LAST_RESULTS
    del temb  # unused by the reference module
    assert not np.any(np.asarray(k_b)), "k bias folds out only when zero"
    x16_r, base, flags = _prep_inputs(x, gn_w, gn_b, q_w, q_b, k_w, k_b,
                                      v_w, v_b, p_w, p_b)
    if flags not in _PROGRAM_CACHE:
        _PROGRAM_CACHE[flags] = _build_program(flags)
    nc = _PROGRAM_CACHE[flags]

    in_maps = [dict(base,
                    xs16=np.ascontiguousarray(x16_r[BPC * i: BPC * (i + 1)]))
               for i in range(NCORES)]
    res = run_bass_kernel_spmd(nc, in_maps, core_ids=list(range(NCORES)))
    LAST_RESULTS = res
    out = np.concatenate([r["out"] for r in res.results], axis=0)
    return np.ascontiguousarray(out.reshape(B, C, H, W).astype(np.float32))
